# revision 35
# baseline (speedup 1.0000x reference)
"""Trainium2 Bass kernel for nn_MEX_41386304864396 (dense transformer block).

Sharding: data-parallel over batch B=8 across 8 NeuronCores (one batch element
per core); weights replicated.  Host pre-transposes activations to [D, S] and
pre-folds constants so the device never transposes:
  * residual fold      W' = I + W                  (embed blocks LN(x + xW + b))
  * LN mean fold       extra matmul column: m = x @ (W'.sum(1)/D) + mean(b)
  * LN affine fold     gamma/beta folded into the downstream q/k/v weights
  * v-bias fold        folded through Wd into the fc bias (softmax rows sum 1)
  * MLP folds          W1 = Wd @ Wfc and W2 = Wproj @ Wml (the Wd and Wml
    matmuls disappear; fc reads attention ctx, proj writes y directly)
  * bridged attention  glb_ctx + plb_ctx = softmax_g @ (vg + softmax_p @ vp)
    (associativity removes the S x S x S 'enhanced' matmul)
Scores are computed transposed [key, query]; softmax uses exp without max
subtraction (scores bounded ~|3.4|) and normalization is deferred:
Z_p is a folded ones-column of the U_p matmul, Z_g a folded ones-row of U_g.
All tensors are bf16 (fp32 PSUM accumulate); LN/softmax scale rows are
broadcast across partitions on the GpSimd engine, keeping the PE free.
Weights stream through one double-buffered SBUF pool as single large DMAs.
"""
import os
import sys

sys.path.insert(0, '/opt/trn_rl_repo')

import numpy as np

import concourse.bass as bass  # noqa: F401
import concourse.tile as tile
from concourse import bacc, mybir
from concourse import bass2jax

F32 = mybir.dt.float32
BF16 = mybir.dt.bfloat16
AF = mybir.ActivationFunctionType
ALU = mybir.AluOpType

S, B, D, H, DH, FF = 512, 8, 1024, 16, 64, 4096
NK = D // 128
NT = S // 128
NFF = FF // 128
NC2 = NFF // 8  # w2 quarter count
EPS = 1e-5
SCALE = 1.0 / 8.0

DT = BF16


def _declare(nc):
    dram = {}

    def din(name, shape, dt=DT):
        dram[name] = nc.dram_tensor(name, list(shape), dt, kind="ExternalInput")

    for n in ("xg", "xl", "xt"):
        din(n, (128, NK, S))
    for n in ("we_l1", "we_l2", "we_g", "w_qg", "w_kg", "w_vg", "w_qp",
              "w_kp", "w_vp"):
        din(n, (128, NK, D))
    din("w_fc", (128, NK, FF))
    din("w_2", (128, NFF, D))
    for e in ("l1", "l2", "g"):
        din(f"wmean_{e}", (128, NK))
        din(f"bcol_{e}", (128, NK), F32)
    for n in ("bcol_qg", "bcol_kg", "bcol_qp", "bcol_kp"):
        din(n, (128, NK), F32)
    din("bcol_fc", (128, NFF), F32)
    din("b2_bc", (128, D), F32)
    dram["y"] = nc.dram_tensor("y", [S, D], F32, kind="ExternalOutput")
    return dram


def _body(nc, tc, dram, mean_b):
    def pool(name, bufs, side="left", space="SBUF"):
        return tc.alloc_tile_pool(name=name, bufs=bufs, side=side, space=space)

    # ---- global pools ----
    consts = pool("consts", 1)
    rows = pool("rows", 1)
    sbB = pool("sbB", 1)
    tmp = pool("tmp", 2)
    sq = pool("sqp", 3)
    small = pool("small", 3)
    outp = pool("outp", 1)
    wp = pool("wp", 1)
    wslot = [0]

    psRow = pool("psRow", 4, space="PSUM")
    psA = pool("psA", 2, space="PSUM")
    psMM = [psA]

    def mmtile():
        return psMM[0].tile([128, 512], F32, tag="mm", name="mm")

    # ---- constants ----
    ones_f = consts.tile([128, 32], F32, tag="ones_f", name="ones_f")
    nc.vector.memset(ones_f[:], 1.0)
    ones_dt = consts.tile([128, 32], DT, tag="ones_dt", name="ones_dt")
    nc.vector.tensor_copy(ones_dt[:], ones_f[:])
    eps_t = consts.tile([1, 1], F32, tag="eps_t", name="eps_t")
    nc.vector.memset(eps_t[:], EPS)

    def cload(name, shape, dt=F32):
        t = consts.tile(list(shape), dt, tag=name)
        nc.sync.dma_start(out=t[:], in_=dram[name].ap())
        return t

    wmean = {e: cload(f"wmean_{e}", (128, NK), DT) for e in ("l1", "l2", "g")}
    bcol_e = {e: cload(f"bcol_{e}", (128, NK)) for e in ("l1", "l2", "g")}
    bcols = {n: cload(f"bcol_{n}", (128, NK))
             for n in ("qg", "kg", "qp", "kp")}
    bcol_fc = cload("bcol_fc", (128, NFF))
    b2_bc = cload("b2_bc", (128, D))

    xp = pool("xp", 1, side="right")
    pe_dup = pool("pe_dup", 1, side="right")
    pe_l2 = pool("pe_l2", 1, side="right")
    pe_g2 = pool("pe_g2", 1, side="right")

    def xload(dname):
        t = xp.tile([128, NK, S], DT, tag=dname, name=dname)
        nc.sync.dma_start(out=t[:], in_=dram[dname].ap())
        return t

    def wload(src_ap, name):
        # explicit 4-way tag cycling: the pool's own rotation reuses the
        # most-recently-freed slot, which defeats multi-phase prefetch
        t = wp.tile([128, NK, D], DT, tag=f"w{wslot[0]}", name=name)
        wslot[0] = (wslot[0] + 1) % 4
        nc.sync.dma_start(out=t[:], in_=src_ap)
        return t

    xg, xl, xt = xload("xg"), xload("xl"), xload("xt")

    # ---- embeds (plain LN; gamma/beta folded downstream on host) ----
    e_state = {}

    def embedA(e, x, wt, pyln):
        mp = psRow.tile([1, 512], F32, tag="row", name="mp")
        for k in range(NK):
            nc.tensor.matmul(mp[:], wmean[e][:, k:k + 1], x[:, k, :],
                             start=(k == 0), stop=(k == NK - 1))
        ss = psRow.tile([1, 512], F32, tag="row", name="ss")
        ys = []
        pend = []
        for m in range(NK):
            ps = mmtile()
            for j in range(NK):
                k = (m + 1 + j) % NK
                nc.tensor.matmul(ps[:], wt[:, k, m * 128:(m + 1) * 128],
                                 x[:, k, :],
                                 start=(j == 0), stop=(j == NK - 1))
            y = pyln.tile([128, 512], DT, tag=f"y{m}", name=f"y{m}")
            nc.vector.tensor_scalar_add(y[:], ps[:], bcol_e[e][:, m:m + 1])
            s = sq.tile([128, 512], DT, tag="sq", name="sq")
            nc.scalar.activation(s[:], ps[:], AF.Square,
                                 bias=bcol_e[e][:, m:m + 1], scale=1.0)
            pend.append(s)
            if m > 0:
                s0 = pend.pop(0)
                nc.tensor.matmul(ss[:], ones_dt[:, 0:1], s0[:],
                                 start=(m == 1), stop=False)
            ys.append(y)
        s0 = pend.pop(0)
        nc.tensor.matmul(ss[:], ones_dt[:, 0:1], s0[:], start=False, stop=True)
        e_state[e] = (mp, ss, ys)

    def embedB(e, epool):
        mp, ss, ys = e_state[e]
        m_sb = rows.tile([1, 512], F32, tag="m_sb", name="m_sb")
        nc.vector.tensor_scalar_add(m_sb[:], mp[:], float(mean_b[e]))
        msq = rows.tile([1, 512], F32, tag="msq", name="msq")
        nc.vector.tensor_mul(msq[:], m_sb[:], m_sb[:])
        var = rows.tile([1, 512], F32, tag="var", name="var")
        nc.vector.scalar_tensor_tensor(out=var[:], in0=ss[:], scalar=1.0 / D,
                                       in1=msq[:], op0=ALU.mult,
                                       op1=ALU.subtract)
        std = rows.tile([1, 512], F32, tag="std", name="std")
        nc.scalar.activation(std[:], var[:], AF.Sqrt, bias=eps_t[:], scale=1.0)
        rstd = rows.tile([1, 512], DT, tag="rstd", name="rstd")
        nc.vector.reciprocal(rstd[:], std[:])
        mr = rows.tile([1, 512], DT, tag="mr", name="mr")
        nc.vector.tensor_mul(mr[:], m_sb[:], rstd[:])
        rb = sbB.tile([128, 512], DT, tag="rb", name="rb")
        nc.gpsimd.partition_broadcast(rb[:], rstd[:])
        mb = sbB.tile([128, 512], DT, tag="mb", name="mb")
        nc.gpsimd.partition_broadcast(mb[:], mr[:])
        et = []
        for m in range(NK):
            t1 = tmp.tile([128, 512], DT, tag="t1", name="t1")
            nc.vector.tensor_mul(t1[:], ys[m][:], rb[:])
            em = epool.tile([128, 512], DT, tag=f"e{m}", name=f"e{m}")
            nc.vector.tensor_sub(em[:], t1[:], mb[:])
            et.append(em)
        e_state[e] = et

    # ---- projections ----
    def projB(wt, src, bcol, opool, tagp):
        out = []
        for m in range(NK):
            ps = mmtile()
            for j in range(NK):
                k = (m + 1 + j) % NK
                nc.tensor.matmul(ps[:], wt[:, k, m * 128:(m + 1) * 128],
                                 src[k], start=(j == 0), stop=(j == NK - 1))
            o = opool.tile([128, 512], DT, tag=f"{tagp}{m}", name=f"{tagp}{m}")
            nc.vector.tensor_scalar_add(o[:], ps[:], bcol[:, m:m + 1])
            out.append(o)
        return out

    def projA(wt, src, opool, tagp, width):
        out = []
        for rt in range(NT):
            vt = opool.tile([128, H, width], DT, tag=f"{tagp}{rt}",
                            name=f"{tagp}{rt}")
            if width == DH + 2:
                for c in (DH, DH + 1):
                    nc.vector.tensor_copy(
                        vt[:, :, c:c + 1].rearrange("p h one -> p (h one)"),
                        ones_dt[:, 0:H])
            for half in range(2):
                ps = mmtile()
                for j in range(NK):
                    k = (2 * rt + half + 1 + j) % NK
                    nc.tensor.matmul(
                        ps[:], src[k][:, rt * 128:(rt + 1) * 128],
                        wt[:, k, half * 512:(half + 1) * 512],
                        start=(j == 0), stop=(j == NK - 1))
                nc.vector.tensor_copy(
                    vt[:, half * 8:(half + 1) * 8, 0:DH],
                    ps[:].rearrange("p (h d) -> p h d", h=8))
            out.append(vt)
        return out

    ext = [xt[:, k, :] for k in range(NK)]

    pqg = pool("pqg", 1)
    pkg = pool("pkg", 1)
    pv = pool("pv", 1)
    pqp = pool("pqp", 1)
    pkp = pool("pkp", 1)
    pyln = pool("pyln", 2)

    # ---- emission: embeds interleaved with the xt-only kp projection ----
    w_l1 = wload(dram["we_l1"].ap(), "we_l1")
    w_l2 = wload(dram["we_l2"].ap(), "we_l2")
    embedA("l1", xl, w_l1, pyln)
    embedA("l2", xl, w_l2, pyln)
    embedB("l1", pe_dup)
    w_g = wload(dram["we_g"].ap(), "we_g")
    embedA("g", xg, w_g, pyln)

    # issue every weight-load config BEFORE any pool release: releases emit
    # SP-queue boundaries that would serialize the prefetch
    w_kp = wload(dram["w_kp"].ap(), "w_kp")
    w_qg = wload(dram["w_qg"].ap(), "w_qg")
    w_kg = wload(dram["w_kg"].ap(), "w_kg")
    w_vg = wload(dram["w_vg"].ap(), "w_vg")
    w_qp = wload(dram["w_qp"].ap(), "w_qp")
    w_vp = wload(dram["w_vp"].ap(), "w_vp")
    fcq = [wload(dram["w_fc"].ap()[:, :, q * 1024:(q + 1) * 1024], f"fcq{q}")
           for q in range(4)]

    # kp projB emitted BEFORE the l2/g LN chains: its PE matmuls need only
    # xt, and its DVE bias-adds queue ahead of the B-chain DVE work, so the
    # psA recycling inside kp never waits on the LN chains
    kpT = projB(w_kp, ext, bcols["kp"], pkp, "kp")
    embedB("l2", pe_l2)
    embedB("g", pe_g2)
    pyln.release()
    dupT, l2T, g2T = e_state["l1"], e_state["l2"], e_state["g"]
    el2 = [t[:] for t in l2T]
    eg2 = [t[:] for t in g2T]
    edup = [t[:] for t in dupT]

    # vp last: its trailing DVE copies overlap attention, which needs no psA
    qgT = projB(w_qg, eg2, bcols["qg"], pqg, "qg")
    pe_g2.release()
    kgT = projB(w_kg, el2, bcols["kg"], pkg, "kg")
    vg = projA(w_vg, el2, pv, "vg", DH)
    pe_l2.release()
    qpT = projB(w_qp, edup, bcols["qp"], pqp, "qp")
    pe_dup.release()
    vp = projA(w_vp, ext, pv, "vpn", DH + 2)
    xp.release()

    # ---- attention (uses no psA: scores->psScore, U_p->psUp, ug->psUg) ----
    psA.release()
    psRow.release()
    psScore = pool("psScore", 2, space="PSUM")
    psUp = pool("psUp", 2, space="PSUM")
    psUg = pool("psUg", 2, space="PSUM")
    pctx = pool("pctx", 1, side="right")
    pexp = pool("pexp", 2, side="right")

    def hsl(tiles, h):
        return tiles[h // 2][64 * (h % 2):64 * (h % 2) + 64, :]

    ctxT = [pctx.tile([128, 512], DT, tag=f"ctx{j}", name=f"ctx{j}")
            for j in range(NK)]
    stage1_out = {}

    def scores_exp(k_h, q_h, tag):
        e = pexp.tile([128, NT, 512], DT, tag=tag, name=tag, bufs=3)
        for pair in range(2):
            sp = psScore.tile([128, 1024], F32, tag="sc", name="sc")
            for i in range(2):
                kt = pair * 2 + i
                nc.tensor.matmul(sp[:, i * 512:(i + 1) * 512],
                                 k_h[:, kt * 128:(kt + 1) * 128], q_h,
                                 start=True, stop=True)
            nc.scalar.activation(
                e[:, 2 * pair:2 * pair + 2, :].rearrange("p a b -> p (a b)"),
                sp[:], AF.Exp, scale=SCALE)
        return e

    def attn_stage1(h):
        qg_h, kg_h = hsl(qgT, h), hsl(kgT, h)
        qp_h, kp_h = hsl(qpT, h), hsl(kpT, h)
        ep = scores_exp(kp_h, qp_h, "ep")
        eg = scores_exp(kg_h, qg_h, "eg")
        up = psUp.tile([128, NT, DH + 2], F32, tag="up", name="up")
        vph = []
        for kt in range(NT):
            for tt in range(NT):
                nc.tensor.matmul(up[:, kt, :],
                                 ep[:, tt, kt * 128:(kt + 1) * 128],
                                 vp[tt][:, h, 0:DH + 2], start=(tt == 0),
                                 stop=(tt == NT - 1))
            rp = small.tile([128, 1], F32, tag="rp", name="rp")
            nc.vector.reciprocal(rp[:], up[:, kt, DH:DH + 1])
            vt = pexp.tile([128, DH + 1], DT, tag=f"vph{kt}", name=f"vph{kt}",
                           bufs=3)
            nc.vector.scalar_tensor_tensor(
                out=vt[:, 0:DH], in0=up[:, kt, 0:DH], scalar=rp[:],
                in1=vg[kt][:, h, :], op0=ALU.mult, op1=ALU.add)
            nc.vector.tensor_copy(vt[:, DH:DH + 1], ones_dt[:, 0:1])
            vph.append(vt)
        stage1_out[h] = (eg, vph)

    def attn_stage2(h):
        eg, vph = stage1_out.pop(h)
        ug = psUg.tile([DH + 1, 512], F32, tag="ug", name="ug")
        for kt in range(NT):
            nc.tensor.matmul(ug[:], vph[kt][:], eg[:, kt, :],
                             start=(kt == 0), stop=(kt == NT - 1))
        rg = rows.tile([1, 512], DT, tag="rg", name="rg")
        nc.vector.reciprocal(rg[:], ug[DH:DH + 1, :])
        rbs = sbB.tile([64, 512], DT, tag="rbs", name="rbs", bufs=2)
        nc.gpsimd.partition_broadcast(rbs[:], rg[:], channels=64)
        off = 64 * (h % 2)
        nc.vector.tensor_mul(ctxT[h // 2][off:off + 64, :], ug[0:DH, :],
                             rbs[:])

    attn_stage1(0)
    attn_stage1(1)
    for h in range(H):
        if h + 2 < H:
            attn_stage1(h + 2)
        attn_stage2(h)
    pexp.release()
    pkp.release()
    pqp.release()
    pv.release()
    pkg.release()
    pqg.release()
    psUg.release()
    psUp.release()
    psScore.release()

    # ---- MLP: h1 = gelu(ctx @ W1 + b1);  y = h1 @ W2 + b2 ----
    psM = pool("psM", 2, space="PSUM")
    psMM[0] = psM
    w2p = pool("w2p", 1)
    ph1 = pool("ph1", 1)
    w2q = []
    for c in range(NC2):
        t = w2p.tile([128, 8, D], DT, tag=f"w2_{c}", name=f"w2_{c}")
        nc.sync.dma_start(out=t[:],
                          in_=dram["w_2"].ap()[:, c * 8:(c + 1) * 8, :])
        w2q.append(t)
    ectx = [t[:] for t in ctxT]
    h1 = []
    for ff in range(NFF):
        wt = fcq[ff // 8]
        ps = mmtile()
        for j in range(NK):
            k = (ff + j) % NK
            nc.tensor.matmul(ps[:],
                             wt[:, k, (ff % 8) * 128:(ff % 8 + 1) * 128],
                             ectx[k], start=(j == 0), stop=(j == NK - 1))
        g = ph1.tile([128, 512], DT, tag=f"h1_{ff}", name=f"h1_{ff}")
        nc.scalar.activation(g[:], ps[:], AF.Gelu,
                             bias=bcol_fc[:, ff:ff + 1], scale=1.0)
        h1.append(g)

    for rt in range(NT):
        yt = outp.tile([128, D], F32, tag="yout", name="yout")
        for half in range(2):
            ps = mmtile()
            for ff in range(NFF):
                nc.tensor.matmul(
                    ps[:], h1[ff][:, rt * 128:(rt + 1) * 128],
                    w2q[ff // 8][:, ff % 8, half * 512:(half + 1) * 512],
                    start=(ff == 0), stop=(ff == NFF - 1))
            nc.vector.tensor_add(yt[:, half * 512:(half + 1) * 512], ps[:],
                                 b2_bc[:, half * 512:(half + 1) * 512])
        nc.sync.dma_start(out=dram["y"].ap()[rt * 128:(rt + 1) * 128, :],
                          in_=yt[:])

    for p in (ph1, w2p, psM, pctx,
              wp, outp, small, sq, tmp, sbB, rows, consts):
        p.release()


def build(repeat=1, mean_b=None):
    mean_b = mean_b or {"l1": 0.0, "l2": 0.0, "g": 0.0}
    nc = bacc.Bacc(None, target_bir_lowering=False, debug=False)
    dram = _declare(nc)
    with tile.TileContext(nc) as tc:
        with nc.allow_low_precision(reason="bf16 storage, fp32 accumulate"):
            if repeat > 1:
                with tc.For_i(0, repeat, 1):
                    _body(nc, tc, dram, mean_b)
            else:
                _body(nc, tc, dram, mean_b)
    nc.compile()

    class CX:
        pass

    cx = CX()
    cx.nc = nc
    cx.dram = dram
    return cx


# ---------------------------------------------------------------------------
# host side
# ---------------------------------------------------------------------------

def _prep_host(inputs):
    f32 = np.float32
    bf = mybir.dt.np(BF16)
    g = np.asarray(inputs["global_feat"], f32)
    l = np.asarray(inputs["local_feat"], f32)
    t = np.asarray(inputs["text_feat"], f32)
    W = {k: np.asarray(inputs[k], f32) for k in
         ("Wg_emb", "Wl1", "Wl2", "Wq_g", "Wk_g", "Wv_g", "Wq_p", "Wk_p",
          "Wv_p", "Wd", "Wml", "Wfc", "Wproj")}
    bv = {k: np.asarray(inputs[k], f32) for k in
          ("bg_emb", "bl1", "bl2", "bq_g", "bk_g", "bv_g", "bq_p", "bk_p",
           "bv_p", "bd", "bml", "bproj", "bfc",
           "betag_emb", "betal1", "betal2", "gg_emb", "gl1", "gl2")}

    def wchunks(Wm):
        # (K, N) -> (128, K//128, N) so a [128, k, :] slice is input-chunk k
        Wm = np.asarray(Wm, f32)
        nch = Wm.shape[0] // 128
        return np.ascontiguousarray(
            Wm.reshape(nch, 128, Wm.shape[1]).transpose(1, 0, 2).astype(bf))

    I = np.eye(D, dtype=f32)
    shared = {}
    mean_b = {}
    for e, (wn, bn) in {"l1": ("Wl1", "bl1"), "l2": ("Wl2", "bl2"),
                        "g": ("Wg_emb", "bg_emb")}.items():
        Wp = (I + W[wn]).astype(f32)
        shared[f"we_{e}"] = wchunks(Wp)
        shared[f"wmean_{e}"] = np.ascontiguousarray(
            (Wp.sum(axis=1) / D).astype(f32).reshape(NK, 128).T.astype(bf))
        mean_b[e] = float(bv[bn].mean())
        shared[f"bcol_{e}"] = np.ascontiguousarray(bv[bn].reshape(NK, 128).T)

    # fold LN gamma/beta of the producing embed into each consumer projection
    def foldp(Wname, bname, gamma, beta):
        Wf = (gamma[:, None] * W[Wname]).astype(f32)
        bf_ = (np.asarray(bv[bname]) + beta @ W[Wname]).astype(f32)
        return Wf, bf_

    w_qg, b_qg = foldp("Wq_g", "bq_g", bv["gg_emb"], bv["betag_emb"])
    w_kg, b_kg = foldp("Wk_g", "bk_g", bv["gl2"], bv["betal2"])
    w_vg, b_vg = foldp("Wv_g", "bv_g", bv["gl2"], bv["betal2"])
    w_qp, b_qp = foldp("Wq_p", "bq_p", bv["gl1"], bv["betal1"])
    shared["w_qg"] = wchunks(w_qg)
    shared["w_kg"] = wchunks(w_kg)
    shared["w_qp"] = wchunks(w_qp)
    shared["w_kp"] = wchunks(W["Wk_p"])
    shared["w_vg"] = wchunks(w_vg)
    shared["w_vp"] = wchunks(W["Wv_p"])
    for n, b_ in (("bcol_qg", b_qg), ("bcol_kg", b_kg), ("bcol_qp", b_qp),
                  ("bcol_kp", bv["bk_p"])):
        shared[n] = np.ascontiguousarray(np.asarray(b_, f32).reshape(NK, 128).T)

    # MLP folds: W1 = Wd@Wfc (fc reads ctx), W2 = Wproj@Wml (proj writes y)
    bdp = (bv["bd"] + (b_vg + bv["bv_p"]) @ W["Wd"]).astype(f32)
    W1 = (W["Wd"] @ W["Wfc"]).astype(f32)
    b1 = (bdp @ W["Wfc"] + bv["bfc"]).astype(f32)
    W2 = (W["Wproj"] @ W["Wml"]).astype(f32)
    b2 = (bv["bproj"] @ W["Wml"] + bv["bml"]).astype(f32)
    shared["w_fc"] = wchunks(W1)
    shared["w_2"] = wchunks(W2)
    shared["bcol_fc"] = np.ascontiguousarray(b1.reshape(NFF, 128).T)
    shared["b2_bc"] = np.ascontiguousarray(
        np.broadcast_to(b2.reshape(1, D), (128, D)))

    def xchunks(xm):
        # [S, D] -> x.T [D, S] -> (128, NK, S)
        return np.ascontiguousarray(
            xm.T.reshape(NK, 128, S).transpose(1, 0, 2).astype(bf))

    in_maps = []
    for b in range(B):
        m = dict(shared)
        m["xg"] = xchunks(g[:, b, :])
        m["xl"] = xchunks(l[:, b, :])
        m["xt"] = xchunks(t[:, b, :])
        in_maps.append(m)
    return in_maps, mean_b


_CACHE = {}


def get_built(repeat, mean_b):
    key = (repeat, tuple(sorted(mean_b.items())))
    if key not in _CACHE:
        _CACHE[key] = build(repeat=repeat, mean_b=mean_b)
    return _CACHE[key]


def run(inputs, repeat=1):
    in_maps, mean_b = _prep_host(inputs)
    cx = get_built(repeat, mean_b)
    results = bass2jax.run_bass_via_pjrt(cx.nc, in_maps, n_cores=B)
    return np.stack([np.asarray(results[b]["y"], np.float32)
                     for b in range(B)], axis=0)


def kernel(**inputs):
    return run(inputs, repeat=int(os.environ.get("BASS_NN_REPEAT", "1")))
